# revision 20
# baseline (speedup 1.0000x reference)
"""ComplexAttention (B=2, T=2048, D=1024, H=16, Dh=64) on 8 TRN2 NeuronCores.

Sharding: core c -> batch b = c // 4, heads [4*(c%4), 4*(c%4)+4).
Each core computes its 4 heads' QKV projections (column-sharded), causal
complex attention, and a partial output projection (row-sharded). The host
sums the 4 partials per batch and adds the output bias.

v3 (f32r rewrite + pipelined epilogue); cost-model 272us/core vs the
fp32 baseline's 1300us. All matmuls stream at 1 cyc/row (fp32 was 4):
  - x is pre-transposed on the host -> no on-device PE transposes.
  - All matmuls are full M=128 f32r (walrus rejects f32r+tile_position and
    f32r M=64 at a PSUM partition offset; M=64/K=1 alone are fine; any
    engine-produced matmul operand must be written with f32r out dtype).
  - Q/K are produced head-PAIR-grouped ([qr_h0;qr_h1]) and regrouped to
    per-head [qr_h;qi_h] (the 128-dim score contraction) via SBUF->SBUF
    partition-shifted engine copies (legal; PSUM-shifted reads are not).
  - V bias is folded into the host-side output bias (attn rows sum to 1),
    so the device never sees it; no Q/K DRAM roundtrip - Q,K,V,O all stay
    SBUF-resident (23 MB peak).
  - O_r/O_i attention matmuls fused into one M=128 matmul per (h,j,k) via
    v_sb's [vr_h(64)|vi_h(64)] free-dim interleave.
  - Attention runs query-block-outer (j), and each block's output
    projection + DMA store is emitted right after its 4 heads, so the
    phase-3 tail pipelines under the next block's attention.
  score = (qr kr^T + qi ki^T)/8 == Qc Kc^T/8, Qc=[qr;qi] (128-d contraction).
  Transposed-domain softmax: S^T[ktok,qtok] tiles, exp on ACT (no max
  subtraction: |S| <~ 3), causal mask via affine_select on Pool,
  row sums l via ones-matmul, 1/l broadcast with a K=1 matmul.
"""

import math
from contextlib import ExitStack

import numpy as np

import concourse.bass as bass
import concourse.tile as tile
from concourse import bacc, mybir
from concourse.bass_utils import run_bass_kernel_spmd

F32 = mybir.dt.float32
USE_F32R = True
F32R = mybir.dt.float32r if USE_F32R else F32

# Full-problem config (hardcoded per harness contract).
CFG = dict(T=2048, D=1024, HPC=4, DH=64, XB=256, QCH=512)
N_CORES = 8
B = 2
H_TOTAL = 16

# Flipped by test.py for profiling; harness path keeps these defaults.
TRACE = False
LAST = {}


def build_program(cfg, num_devices=N_CORES, enable_asserts=False, repeat=1,
                  phases="123"):
    """Build the per-core SPMD Bass program. repeat>1 replicates the whole
    body (for in-NEFF loop timing). phases is a dev aid to time subsets."""
    nc = bacc.Bacc(
        "TRN2",
        target_bir_lowering=False,
        debug=False,
        enable_asserts=enable_asserts,
        num_devices=num_devices,
    )
    T, D, HPC, DH = cfg["T"], cfg["D"], cfg["HPC"], cfg["DH"]
    CW = HPC * DH
    P = 128

    # ---- DRAM I/O ----
    x_rT = nc.dram_tensor("x_rT", [D, T], F32, kind="ExternalInput").ap()
    x_iT = nc.dram_tensor("x_iT", [D, T], F32, kind="ExternalInput").ap()
    wq_r = nc.dram_tensor("wq_r", [D, CW], F32, kind="ExternalInput").ap()
    wq_i = nc.dram_tensor("wq_i", [D, CW], F32, kind="ExternalInput").ap()
    wk_r = nc.dram_tensor("wk_r", [D, CW], F32, kind="ExternalInput").ap()
    wk_i = nc.dram_tensor("wk_i", [D, CW], F32, kind="ExternalInput").ap()
    wv_r = nc.dram_tensor("wv_r", [D, CW], F32, kind="ExternalInput").ap()
    wv_i = nc.dram_tensor("wv_i", [D, CW], F32, kind="ExternalInput").ap()
    wo_r = nc.dram_tensor("wo_r", [CW, D], F32, kind="ExternalInput").ap()
    wo_i = nc.dram_tensor("wo_i", [CW, D], F32, kind="ExternalInput").ap()
    bq = nc.dram_tensor("bq", [P, HPC], F32, kind="ExternalInput").ap()
    bk = nc.dram_tensor("bk", [P, HPC], F32, kind="ExternalInput").ap()
    bq_sw = nc.dram_tensor("bq_sw", [P, HPC], F32, kind="ExternalInput").ap()
    bk_sw = nc.dram_tensor("bk_sw", [P, HPC], F32, kind="ExternalInput").ap()
    out_r = nc.dram_tensor("out_r", [T, D], F32, kind="ExternalOutput").ap()
    out_i = nc.dram_tensor("out_i", [T, D], F32, kind="ExternalOutput").ap()

    with tile.TileContext(nc) as tc:
        for _ in range(repeat):
            _body(nc, tc, cfg,
                  (x_rT, x_iT, wq_r, wq_i, wk_r, wk_i, wv_r, wv_i,
                   wo_r, wo_i, bq, bk, bq_sw, bk_sw, out_r, out_i),
                  phases=phases)

    nc.compile()
    return nc


def _body(nc, tc, cfg, aps, phases="123"):
    (x_rT, x_iT, wq_r, wq_i, wk_r, wk_i, wv_r, wv_i,
     wo_r, wo_i, bq, bk, bq_sw, bk_sw, out_r, out_i) = aps
    T, D, HPC, DH = cfg["T"], cfg["D"], cfg["HPC"], cfg["DH"]
    XB, QCH = cfg["XB"], cfg["QCH"]
    P = 128
    DT = D // P            # d tiles
    NBLK = T // XB         # phase-1 token blocks
    SK = XB // P           # key tiles per block
    KT = T // P            # key tiles
    QC = T // QCH          # phase-2 query chunks
    QKB = QCH // P         # key tiles per query chunk step
    CW = HPC * DH
    NPAIR = HPC // 2
    scale = 1.0 / math.sqrt(DH)
    assert DH == 64 and CW == 2 * P

    R = lambda ap: ap.bitcast(F32R)

    x_rT_t = x_rT.rearrange("(t p) m -> p t m", p=P)
    x_iT_t = x_iT.rearrange("(t p) m -> p t m", p=P)
    out_r_t = out_r.rearrange("(n p) d -> p n d", p=P)
    out_i_t = out_i.rearrange("(n p) d -> p n d", p=P)

    with ExitStack() as octx:
        const = octx.enter_context(tc.tile_pool(name="const", bufs=1))
        opool = octx.enter_context(tc.tile_pool(name="opool", bufs=1))

        # memset can't write f32r; stage f32 ones and cast via ACT copy
        ones_st = const.tile([P, P], F32)
        nc.vector.memset(ones_st, 1.0)
        ones_col = const.tile([P, 1], F32R)   # lhsT for l = ones^T @ expS
        nc.scalar.activation(ones_col, ones_st[:, 0:1],
                             mybir.ActivationFunctionType.Copy)
        ones_row = const.tile([1, P], F32R)   # lhsT for 1/l broadcast
        nc.scalar.activation(ones_row, ones_st[0:1, :],
                             mybir.ActivationFunctionType.Copy)
        bq_sb = const.tile([P, HPC], F32)
        nc.sync.dma_start(bq_sb, bq)
        bk_sb = const.tile([P, HPC], F32)
        nc.sync.dma_start(bk_sb, bk)
        bqsw_sb = const.tile([P, HPC], F32)
        nc.sync.dma_start(bqsw_sb, bq_sw)
        bksw_sb = const.tile([P, HPC], F32)
        nc.sync.dma_start(bksw_sb, bk_sw)

        # V SBUF-resident: [tok_p, ktile, h*128 + (vr64|vi64)]
        v_sb = opool.tile([P, KT, HPC * P], F32R)
        # Qc/Kc per head, d-major: rows [qr_h(64); qi_h(64)], cols = tokens
        qc = [opool.tile([P, T], F32R, name=f"qc{h}") for h in range(HPC)]
        kc = [opool.tile([P, T], F32R, name=f"kc{h}") for h in range(HPC)]

        # ================= Phase 1: projections =================
        if "1" not in phases:
            return
        with ExitStack() as ctx:
            wpool = ctx.enter_context(tc.tile_pool(name="wpool", bufs=1))
            xin = ctx.enter_context(tc.tile_pool(name="xin", bufs=2))
            stg = ctx.enter_context(tc.tile_pool(name="stg", bufs=3))
            ps_qk = ctx.enter_context(
                tc.tile_pool(name="ps_qk", bufs=3, space="PSUM"))
            ps_v = ctx.enter_context(
                tc.tile_pool(name="ps_v", bufs=2, space="PSUM"))

            def load_w(ap_dram, name):
                # two tiles (lo/hi halves of d) so the first block's matmuls
                # start after half the weight lands
                hd = DT // 2
                view = ap_dram.rearrange("(t p) m -> p t m", p=P)
                lo = wpool.tile([P, hd, CW], F32R, name=name + "_lo")
                nc.sync.dma_start(lo, R(view[:, 0:hd, :]))
                hi = wpool.tile([P, hd, CW], F32R, name=name + "_hi")
                nc.sync.dma_start(hi, R(view[:, hd:DT, :]))

                class WPair:
                    def __getitem__(self, idx):
                        _, d, c = idx
                        return (lo if d < hd else hi)[:, d % hd, c]
                return WPair()

            wq_r_sb = load_w(wq_r, "wq_r_sb")
            wq_i_sb = load_w(wq_i, "wq_i_sb")
            wk_r_sb = load_w(wk_r, "wk_r_sb")
            wk_i_sb = load_w(wk_i, "wk_i_sb")
            wv_r_sb = load_w(wv_r, "wv_r_sb")
            wv_i_sb = load_w(wv_i, "wv_i_sb")

            for blk in range(NBLK):
                cs = slice(blk * XB, (blk + 1) * XB)
                xr_c = xin.tile([P, DT, XB], F32R, name="xr_c")
                nc.sync.dma_start(xr_c, R(x_rT_t[:, :, cs]))
                xi_c = xin.tile([P, DT, XB], F32R, name="xi_c")
                nc.sync.dma_start(xi_c, R(x_iT_t[:, :, cs]))

                # Q/K head-pair-grouped psums, then regroup to per-head qc/kc
                for pair in range(NPAIR):
                    h0, h1 = 2 * pair, 2 * pair + 1
                    pc = slice(pair * P, (pair + 1) * P)
                    for (wA, wB, bias, bias_sw, dst) in (
                        (wq_r_sb, wq_i_sb, bq_sb, bqsw_sb, qc),
                        (wk_r_sb, wk_i_sb, bk_sb, bksw_sb, kc),
                    ):
                        psA = ps_qk.tile([P, XB], F32, name="psA", tag="psA")
                        psB = ps_qk.tile([P, XB], F32, name="psB", tag="psB")
                        for d in range(DT):
                            nc.tensor.matmul(
                                psA, wA[:, d, pc], xr_c[:, d, :],
                                start=(d == 0), stop=(d == DT - 1))
                            nc.tensor.matmul(
                                psB, wB[:, d, pc], xi_c[:, d, :],
                                start=(d == 0), stop=(d == DT - 1))
                        # psA = [Ar_h0; Ar_h1], psB = [Ai_h0; Ai_h1]
                        # direct same-partition halves (bias fused):
                        nc.any.tensor_scalar_add(
                            out=dst[h0][0:64, cs], in0=psA[0:64],
                            scalar1=bias[0:64, h0:h0 + 1])
                        nc.any.tensor_scalar_add(
                            out=dst[h1][64:128, cs], in0=psB[64:128],
                            scalar1=bias[64:128, h1:h1 + 1])
                        # crossing halves: stage (bias fused), then
                        # partition-shifted SBUF->SBUF copies
                        st = stg.tile([P, XB], F32R, name="st_qk")
                        nc.any.tensor_scalar_add(
                            out=st[64:128], in0=psA[64:128],
                            scalar1=bias_sw[64:128, h1:h1 + 1])
                        nc.any.tensor_scalar_add(
                            out=st[0:64], in0=psB[0:64],
                            scalar1=bias_sw[0:64, h0:h0 + 1])
                        nc.any.tensor_copy(
                            out=dst[h1][0:64, cs], in_=st[64:128])
                        nc.any.tensor_copy(
                            out=dst[h0][64:128, cs], in_=st[0:64])

                # V token-major: psum [tok(128), CW] r and i, pack interleaved
                for s in range(SK):
                    ktile = blk * SK + s
                    ts = slice(s * P, (s + 1) * P)
                    pvr = ps_v.tile([P, CW], F32, name="pvr", tag="pv")
                    pvi = ps_v.tile([P, CW], F32, name="pvi", tag="pv")
                    for d in range(DT):
                        nc.tensor.matmul(
                            pvr, xr_c[:, d, ts], wv_r_sb[:, d, :],
                            start=(d == 0), stop=(d == DT - 1))
                        nc.tensor.matmul(
                            pvi, xi_c[:, d, ts], wv_i_sb[:, d, :],
                            start=(d == 0), stop=(d == DT - 1))
                    for h in range(HPC):
                        nc.any.tensor_copy(
                            out=v_sb[:, ktile, h * P:h * P + 64],
                            in_=pvr[:, h * DH:(h + 1) * DH])
                        nc.any.tensor_copy(
                            out=v_sb[:, ktile, h * P + 64:(h + 1) * P],
                            in_=pvi[:, h * DH:(h + 1) * DH])

        # ============ Phase 2+3: attention + output projection ============
        # j-outer: all 4 heads' attention for query block j, then that
        # block's regroup + output projection + store, so phase-3 work and
        # output DMA pipeline under the next block's attention.
        if "2" not in phases:
            return
        with ExitStack() as ctx23:
            ohp = ctx23.enter_context(tc.tile_pool(name="ohp", bufs=1))
            # per-head O^T, rows [o_r(64); o_i(64)]
            oh = [ohp.tile([P, T], F32R, name=f"oh{h}") for h in range(HPC)]

            with ExitStack() as ctx:
                wop = ctx.enter_context(tc.tile_pool(name="wop", bufs=1))
                ogp = ctx.enter_context(tc.tile_pool(name="ogp", bufs=1))
                epool = ctx.enter_context(tc.tile_pool(name="epool", bufs=6))
                rpool = ctx.enter_context(tc.tile_pool(name="rpool", bufs=2))
                sout = ctx.enter_context(tc.tile_pool(name="sout", bufs=3))
                ps_s = ctx.enter_context(
                    tc.tile_pool(name="ps_s", bufs=3, space="PSUM"))
                ps_o = ctx.enter_context(
                    tc.tile_pool(name="ps_o", bufs=2, space="PSUM"))
                ps_lb = ctx.enter_context(
                    tc.tile_pool(name="ps_lb", bufs=1, space="PSUM"))
                ps_f = ctx.enter_context(
                    tc.tile_pool(name="ps_f", bufs=1, space="PSUM"))

                wor_sb = wop.tile([P, NPAIR, D], F32R, name="wor_sb")
                nc.sync.dma_start(
                    wor_sb, R(wo_r.rearrange("(t p) m -> p t m", p=P)))
                woi_sb = wop.tile([P, NPAIR, D], F32R, name="woi_sb")
                nc.sync.dma_start(
                    woi_sb, R(wo_i.rearrange("(t p) m -> p t m", p=P)))

                # regrouped O: orP[pair] = [or_h0; or_h1], oiP likewise
                orP = [ogp.tile([P, T], F32R, name=f"orP{p}")
                       for p in range(NPAIR)]
                oiP = [ogp.tile([P, T], F32R, name=f"oiP{p}")
                       for p in range(NPAIR)]
                NC2 = D // QCH

                for j in range(QC):
                    nk = (j + 1) * QKB
                    qs = slice(j * QCH, (j + 1) * QCH)
                    for h in range(HPC):
                        po = ps_o.tile([P, QCH], F32, name="po")
                        pl = ps_lb.tile([1, QCH], F32, name="pl")
                        for k in range(nk):
                            st = ps_s.tile([P, QCH], F32, name="st")
                            nc.tensor.matmul(
                                st, kc[h][:, k * P:(k + 1) * P],
                                qc[h][:, qs], start=True, stop=True)
                            et = epool.tile([P, QCH], F32R, name="et")
                            nc.scalar.activation(
                                et, st, mybir.ActivationFunctionType.Exp,
                                scale=scale)
                            if k >= j * QKB:
                                # keep where qtok >= ktok:
                                #   -p + f + (QCH*j - 128*k) >= 0
                                nc.gpsimd.affine_select(
                                    out=et, in_=et,
                                    compare_op=mybir.AluOpType.is_ge,
                                    fill=0.0,
                                    base=QCH * j - P * k,
                                    pattern=[[1, QCH]],
                                    channel_multiplier=-1)
                            nc.tensor.matmul(
                                pl, ones_col, et,
                                start=(k == 0), stop=(k == nk - 1))
                            nc.tensor.matmul(
                                po, v_sb[:, k, h * P:(h + 1) * P], et,
                                start=(k == 0), stop=(k == nk - 1))
                        rl = rpool.tile([1, QCH], F32R, name="rl")
                        with nc.allow_low_precision(
                                reason="1/l in f32r feeds f32r bcast matmul"):
                            nc.vector.reciprocal(rl, pl)
                        pb = ps_lb.tile([P, QCH], F32, name="pb")
                        nc.tensor.matmul(pb, ones_row, rl,
                                         start=True, stop=True)
                        sb_b = rpool.tile([P, QCH], F32, name="sb_b")
                        nc.any.tensor_copy(out=sb_b, in_=pb)
                        nc.any.tensor_mul(out=oh[h][:, qs], in0=po,
                                          in1=sb_b)

                    # ---- phase 3 for this query block ----
                    if "3" not in phases:
                        continue
                    for pair in range(NPAIR):
                        h0, h1 = 2 * pair, 2 * pair + 1
                        nc.any.tensor_copy(out=orP[pair][0:64, qs],
                                           in_=oh[h0][0:64, qs])
                        nc.any.tensor_copy(out=orP[pair][64:128, qs],
                                           in_=oh[h1][0:64, qs])
                        nc.any.tensor_copy(out=oiP[pair][0:64, qs],
                                           in_=oh[h0][64:128, qs])
                        nc.any.tensor_copy(out=oiP[pair][64:128, qs],
                                           in_=oh[h1][64:128, qs])
                    for (oP, wsb, odst) in (
                        (orP, wor_sb, out_r_t), (oiP, woi_sb, out_i_t)
                    ):
                        for t in range(j * QCH // P, (j + 1) * QCH // P):
                            for n in range(NC2):
                                pf = ps_f.tile([P, QCH], F32, name="pf")
                                for kk in range(NPAIR):
                                    nc.tensor.matmul(
                                        pf,
                                        oP[kk][:, t * P:(t + 1) * P],
                                        wsb[:, kk, n * QCH:(n + 1) * QCH],
                                        start=(kk == 0),
                                        stop=(kk == NPAIR - 1))
                                ot = sout.tile([P, QCH], F32, name="ot")
                                nc.any.tensor_copy(out=ot, in_=pf)
                                nc.sync.dma_start(
                                    odst[:, t, n * QCH:(n + 1) * QCH], ot)


def make_core_inputs(inputs, cfg=CFG):
    """Slice full inputs into 8 per-core input maps."""
    HPC, DH = cfg["HPC"], cfg["DH"]
    CW = HPC * DH
    f = lambda a: np.ascontiguousarray(np.asarray(a, dtype=np.float32))
    x_rT = [f(np.asarray(inputs["x_real"][b]).T) for b in range(B)]
    x_iT = [f(np.asarray(inputs["x_imag"][b]).T) for b in range(B)]
    maps = []
    for c in range(N_CORES):
        b = c // 4
        g = c % 4
        cs = slice(g * CW, (g + 1) * CW)
        bqr, bqi = f(inputs["bqr"])[cs], f(inputs["bqi"])[cs]
        bkr, bki = f(inputs["bkr"])[cs], f(inputs["bki"])[cs]
        pack = lambda br, bi: np.stack(
            [np.concatenate([br[h * DH:(h + 1) * DH],
                             bi[h * DH:(h + 1) * DH]])
             for h in range(HPC)], axis=1)
        maps.append({
            "x_rT": x_rT[b], "x_iT": x_iT[b],
            "wq_r": f(inputs["Wqr"])[:, cs], "wq_i": f(inputs["Wqi"])[:, cs],
            "wk_r": f(inputs["Wkr"])[:, cs], "wk_i": f(inputs["Wki"])[:, cs],
            "wv_r": f(inputs["Wvr"])[:, cs], "wv_i": f(inputs["Wvi"])[:, cs],
            "wo_r": f(inputs["Wor"])[cs, :], "wo_i": f(inputs["Woi"])[cs, :],
            "bq": np.ascontiguousarray(pack(bqr, bqi)),
            "bk": np.ascontiguousarray(pack(bkr, bki)),
            "bq_sw": np.ascontiguousarray(pack(bqi, bqr)),
            "bk_sw": np.ascontiguousarray(pack(bki, bkr)),
        })
    return maps


def kernel(**inputs):
    global LAST
    nc = build_program(CFG)
    in_maps = make_core_inputs(inputs)
    res = run_bass_kernel_spmd(
        nc, in_maps, core_ids=list(range(N_CORES)), trace=TRACE)
    LAST = {"exec_time_ns": res.exec_time_ns,
            "trace": res.instructions_and_trace,
            "profile_json": res.profile_json,
            "nc": nc}
    f = lambda a: np.asarray(a, dtype=np.float32)
    # V bias folded here: attn rows sum to 1 -> adds (bv @ Wo) to every row.
    bor_eff = f(inputs["bor"]) + f(inputs["bvr"]) @ f(inputs["Wor"])
    boi_eff = f(inputs["boi"]) + f(inputs["bvi"]) @ f(inputs["Woi"])
    final_r = np.stack([
        sum(res.results[c]["out_r"] for c in range(4 * b, 4 * b + 4))
        + bor_eff for b in range(B)]).astype(np.float32)
    final_i = np.stack([
        sum(res.results[c]["out_i"] for c in range(4 * b, 4 * b + 4))
        + boi_eff for b in range(B)]).astype(np.float32)
    return final_r, final_i
